# revision 28
# baseline (speedup 1.0000x reference)
"""Batch Child-Sum TreeLSTM cell on 8 Trainium2 NeuronCores.

Strategy (data-parallel over nodes; ~231-249us/core at full size):
  - Shard the N nodes (and their contiguous child segments) evenly across the
    8 cores; replicate the small weight matrices. Irregular sorted
    segment_ids are first regularized host-side by zero-padding every node to
    max_children slots (exact: padded slots contribute 0).
  - Host stages activations feature-major ("transposed": features on SBUF
    partitions) and child-major (one contiguous slab per child slot), cast to
    fp16. Every matmul contraction runs along partitions, every moving
    operand is a contiguous slice, segment sums are plain slab adds on the
    free axis, and HBM reads halve vs fp32.
  - fp16 (not bf16): same PE/DVE throughput, 10-bit mantissa; all values here
    are O(6) so fp16 is strictly more accurate. End-to-end rel err ~2e-3.
  - PE streams each sub-tile's big_in once per z chunk (h_tilde materialized
    by two DVE fp16 2x-mode adds); forget gates are computed child-major so
    the per-edge U_f h term and the replicated W_f x term are plain matmuls.
  - ScalarE evacuates every PSUM with the fused bias+sigmoid/tanh ACTIVATE;
    VectorE does all gate arithmetic in fp16 packed mode; outputs store as
    fp16 and the host upcasts. Output stores ride the idle gpsimd SWDGE
    queue so the next macro's loads never queue behind them in the SP HWDGE
    FIFO; all three forget-gate slots accumulate into one 3-bank PSUM tile
    (512-aligned slices) evacuated by a single bias-fused sigmoid; the first
    and last macro tiles are small so compute starts and stores finish early.
    Engine busy: PE/ACT/DMA ~195-215us each, DVE ~190us.
"""

from contextlib import ExitStack

import numpy as np

import concourse.bass as bass
import concourse.bacc as bacc
import concourse.tile as tile
from concourse import mybir
from concourse.bass_utils import run_bass_kernel_spmd

F32 = mybir.dt.float32
F32R = mybir.dt.float32r
BF16 = mybir.dt.bfloat16
FP16 = mybir.dt.float16

# Matmul operand dtype. "bf16": x/child_h are cast to bf16 during the load
# DMA (SWDGE) and weights are passed as bf16 -> single-pass matmuls at full
# PE rate. "fp32r": everything stays 32-bit (2-pass matmuls, ~2x PE time)
# but ~bf16x2 precision.
MM_DT = "bf16"

# When True, the whole gate chain (activations, forget gates, cell/hidden
# assembly) and the outputs run in the 16-bit dtype: DVE tensor_tensor gets
# its 2x packed mode and the output DMA halves.
GATES_BF16 = True

# Use fp16 (not bf16) as the 16-bit dtype: same PE/DVE rates, but a 10-bit
# mantissa. All values here are O(6), nowhere near fp16 range limits, so
# fp16 is strictly more accurate than bf16 for this kernel.
LOWP_FP16 = True

# Optionally keep the h-side (sig_o, tanh_c, h) in fp32.
H_F32 = False

# Materialize h_tilde with two DVE adds instead of three extra accumulating
# matmuls per z chunk: PE streams each sub's big_in once less (15->9 MMs/sub),
# and DVE's fp16 2x mode makes the adds cheap.
HT_ON_DVE = True

N_CORES = 8

# Tiling (in nodes). MACRO: DMA / SBUF-residency granularity for x, child
# slabs and outputs. SUB: matmul moving free dim / PSUM tile width (<= 512
# fp32 per PSUM bank; keep even — walrus's fp32r dst-pattern check rejects
# odd free dims on the fp32r fallback path).
MACRO = 2500
SUB = 500


def _chunks(total, step):
    out = []
    off = 0
    while off < total:
        out.append((off, min(step, total - off)))
        off += step
    return out


def build_program(npc, in_dim, hid, cpn, engines=None):
    """Bass program for one core's shard: npc nodes, npc*cpn edges."""
    assert in_dim % 128 == 0 and hid == 128
    kx = in_dim // 128  # k-chunks of the input features
    epc = npc * cpn

    # gpsimd is poison for fine-grained work here: its EVENT_SEMAPHORE ops are
    # ucode-dispatched (~1.3us each) and dominate its engine span.
    eng = {"fjc": "vector", "fc": "vector", "gates": "vector"}
    if engines:
        eng.update(engines)

    mm_bf16 = MM_DT == "bf16"
    LP = FP16 if LOWP_FP16 else BF16  # low-precision dtype
    MMDT = LP if mm_bf16 else F32R
    WDT = LP if mm_bf16 else F32R
    GDT = LP if GATES_BF16 else F32   # gate-chain dtype
    ODT = LP if GATES_BF16 else F32   # c output dtype
    HDT = F32 if (H_F32 or not GATES_BF16) else LP  # h-side dtype

    nc = bacc.Bacc("TRN2", target_bir_lowering=False, debug=False)

    xT = nc.dram_tensor("xT", [in_dim, npc], MMDT if mm_bf16 else F32R, kind="ExternalInput").ap()
    chT = nc.dram_tensor("chT", [hid, epc], MMDT if mm_bf16 else F32R, kind="ExternalInput").ap()
    ccT = nc.dram_tensor("ccT", [hid, epc], GDT, kind="ExternalInput").ap()
    # W_combined [in+hid, 3*hid] row-major, sliced into [128,128] chunks.
    Wc = nc.dram_tensor("Wc", [in_dim + hid, 3 * hid], WDT, kind="ExternalInput").ap()
    Wf = nc.dram_tensor("Wf", [in_dim, hid], WDT, kind="ExternalInput").ap()
    Uf = nc.dram_tensor("Uf", [hid, hid], WDT, kind="ExternalInput").ap()
    bc = nc.dram_tensor("bc", [hid, 3], F32, kind="ExternalInput").ap()  # b_combined.reshape(3,128).T
    bf = nc.dram_tensor("bf", [hid, 1], F32, kind="ExternalInput").ap()

    chT3 = chT.rearrange("p (c n) -> p c n", c=cpn)
    ccT3 = ccT.rearrange("p (c n) -> p c n", c=cpn)

    cT = nc.dram_tensor("cT", [hid, npc], ODT, kind="ExternalOutput").ap()
    hT = nc.dram_tensor("hT", [hid, npc], HDT, kind="ExternalOutput").ap()

    with tile.TileContext(nc) as tc, ExitStack() as ctx:
        consts = ctx.enter_context(tc.tile_pool(name="consts", bufs=1))
        macro_pool = ctx.enter_context(tc.tile_pool(name="macro", bufs=2))
        work = ctx.enter_context(tc.tile_pool(name="work", bufs=3))
        psum = ctx.enter_context(tc.tile_pool(name="psum", bufs=2, space="PSUM"))

        # ---- weights (resident) ----
        # wc_sb[k] holds Wc rows [128k:128k+128]; k in [0, kx] are x chunks,
        # k == kx is the h_tilde chunk. Columns: 3*hid (z_i | z_o | z_u).
        wc_sb = []
        for k in range(kx + 1):
            t = consts.tile([128, 3 * hid], WDT, tag=f"wc{k}")
            nc.sync.dma_start(out=t, in_=Wc[128 * k : 128 * (k + 1), :])
            wc_sb.append(t)
        wf_sb = []
        for k in range(kx):
            t = consts.tile([128, hid], WDT, tag=f"wf{k}")
            nc.sync.dma_start(out=t, in_=Wf[128 * k : 128 * (k + 1), :])
            wf_sb.append(t)
        uf_sb = consts.tile([128, hid], WDT, tag="uf")
        nc.sync.dma_start(out=uf_sb, in_=Uf)
        bc_sb = consts.tile([128, 3], F32, tag="bc")
        nc.sync.dma_start(out=bc_sb, in_=bc)
        bf_sb = consts.tile([128, 1], F32, tag="bf")
        nc.sync.dma_start(out=bf_sb, in_=bf)

        ACTF = mybir.ActivationFunctionType
        # Shared 3-bank PSUM tile for all forget-gate slots (single sigmoid
        # evacuation). Allocated once (bufs=1 semantics) and memset so the
        # pad columns between the slots are never read uninitialized.
        fp_all = None
        if cpn >= 2 and SUB <= 512 and cpn <= 3:
            fp_all = psum.tile([128, 512 * cpn], F32, tag="fp_all", bufs=1)
            nc.vector.memset(fp_all, 0.0)
        fjc_eng = getattr(nc, eng["fjc"])
        fc_eng = getattr(nc, eng["fc"])
        gate_eng = getattr(nc, eng["gates"])

        macro_plan = []
        if npc > 2 * MACRO:
            # small first macro (compute starts early) and small last macro
            # (stores start early, short tail)
            macro_plan.append((0, SUB))
            macro_plan += [(SUB + o, s) for o, s in _chunks(npc - 2 * SUB, MACRO)]
            macro_plan.append((npc - SUB, SUB))
        else:
            macro_plan = _chunks(npc, MACRO)
        for m0, msz in macro_plan:
            x_sb = []
            for k in range(kx):
                t = macro_pool.tile([128, msz], MMDT, tag=f"x{k}", bufs=3)
                nc.sync.dma_start(out=t, in_=xT[128 * k : 128 * (k + 1), m0 : m0 + msz])
                x_sb.append(t)
            c_out = macro_pool.tile([128, msz], ODT, tag="c_out")
            h_out = macro_pool.tile([128, msz], HDT, tag="h_out")

            # child tensors: one contiguous DMA per child slab per macro
            ch_mac = macro_pool.tile([128, cpn * msz], MMDT, tag="ch", bufs=3)
            cc_mac = macro_pool.tile([128, cpn * msz], GDT, tag="cc", bufs=3)
            for c in range(cpn):
                nc.sync.dma_start(
                    out=ch_mac[:, c * msz : (c + 1) * msz],
                    in_=chT3[:, c, m0 : m0 + msz],
                )
                nc.sync.dma_start(
                    out=cc_mac[:, c * msz : (c + 1) * msz],
                    in_=ccT3[:, c, m0 : m0 + msz],
                )
            ch_mac3 = ch_mac.rearrange("p (c n) -> p c n", c=cpn)
            cc_mac3 = cc_mac.rearrange("p (c n) -> p c n", c=cpn)

            for s0, ssz in _chunks(msz, SUB):
                n0 = m0 + s0  # absolute node offset
                esz = ssz * cpn

                ch3 = ch_mac3[:, :, s0 : s0 + ssz]
                cc3 = cc_mac3[:, :, s0 : s0 + ssz]

                if HT_ON_DVE and cpn > 1:
                    ht = work.tile([128, ssz], MMDT, tag="ht")
                    nc.vector.tensor_add(ht, ch3[:, 0, :], ch3[:, 1, :])
                    for ci in range(2, cpn):
                        nc.vector.tensor_add(ht, ht, ch3[:, ci, :])
                elif HT_ON_DVE:
                    ht = ch3[:, 0, :]

                # ---- z = big_in @ W_combined + b_combined (transposed) ----
                zp = []
                for j in range(3):  # z_i, z_o, z_u output chunks
                    p = psum.tile([128, ssz], F32, tag=f"z{j}", bufs=(1 if j == 2 else 2))
                    for k in range(kx):
                        nc.tensor.matmul(
                            p,
                            lhsT=wc_sb[k][:, 128 * j : 128 * (j + 1)],
                            rhs=x_sb[k][:, s0 : s0 + ssz],
                            start=(k == 0),
                            stop=False,
                        )
                    if HT_ON_DVE:
                        nc.tensor.matmul(
                            p,
                            lhsT=wc_sb[kx][:, 128 * j : 128 * (j + 1)],
                            rhs=ht,
                            start=False,
                            stop=True,
                        )
                    else:
                        for c in range(cpn):  # += W_h^T @ h_tilde^T
                            nc.tensor.matmul(
                                p,
                                lhsT=wc_sb[kx][:, 128 * j : 128 * (j + 1)],
                                rhs=ch3[:, c, :],
                                start=False,
                                stop=(c == cpn - 1),
                            )
                    zp.append(p)

                sig_i = work.tile([128, ssz], GDT, tag="sig_i")
                nc.scalar.activation(sig_i, zp[0], ACTF.Sigmoid, bias=bc_sb[:, 0:1])
                sig_o = work.tile([128, ssz], HDT, tag="sig_o")
                nc.scalar.activation(sig_o, zp[1], ACTF.Sigmoid, bias=bc_sb[:, 1:2])
                tanh_u = work.tile([128, ssz], GDT, tag="tanh_u")
                nc.scalar.activation(tanh_u, zp[2], ACTF.Tanh, bias=bc_sb[:, 2:3])

                # ---- f_jk = sigmoid(W_f x_j + U_f h_k + b_f), child-major ----
                # One PSUM tile per child slot c: f_c = U_f^T ch[:, c::cpn]
                # + W_f^T x (x term identical for all slots, plain slices —
                # no broadcast APs, which the fp32r ISA checker rejects).
                merged = fp_all is not None and ssz <= 512
                fw = 512 * (cpn - 1) + ssz
                if merged:
                    f_all = work.tile([128, fw], GDT, tag="f_all")
                f_slices = []
                for c in range(cpn):
                    if merged:
                        fp = fp_all[:, 512 * c : 512 * c + ssz]
                    else:
                        fp = psum.tile([128, ssz], F32, tag="fp", bufs=1)
                    nc.tensor.matmul(
                        fp, lhsT=uf_sb, rhs=ch3[:, c, :], start=True, stop=False
                    )
                    for k in range(kx):
                        nc.tensor.matmul(
                            fp,
                            lhsT=wf_sb[k],
                            rhs=x_sb[k][:, s0 : s0 + ssz],
                            start=False,
                            stop=(k == kx - 1),
                        )
                    if merged:
                        f_slices.append(f_all[:, 512 * c : 512 * c + ssz])
                        if c == cpn - 1:
                            # single sigmoid evacuates every slot (same b_f
                            # bias); pad columns hold garbage nobody reads
                            nc.scalar.activation(
                                f_all, fp_all[:, 0:fw], ACTF.Sigmoid,
                                bias=bf_sb[:, 0:1],
                            )
                        continue
                    f_c = work.tile([128, ssz], GDT, tag=f"f{c}")
                    nc.scalar.activation(f_c, fp, ACTF.Sigmoid, bias=bf_sb[:, 0:1])
                    f_slices.append(f_c)
                fjc_c = []
                for c in range(cpn):
                    fjc = work.tile([128, ssz], GDT, tag=f"fjc{c}")
                    fjc_eng.tensor_mul(fjc, f_slices[c], cc3[:, c, :])
                    fjc_c.append(fjc)

                # ---- fc = segment_sum(f * child_c) ----
                if cpn == 1:
                    fc = fjc_c[0]
                else:
                    fc = work.tile([128, ssz], GDT, tag="fc")
                    fc_eng.tensor_add(fc, fjc_c[0], fjc_c[1])
                    for ci in range(2, cpn):
                        fc_eng.tensor_add(fc, fc, fjc_c[ci])

                # ---- c, h ----
                c_sl = c_out[:, s0 : s0 + ssz]
                h_sl = h_out[:, s0 : s0 + ssz]
                gate_eng.tensor_mul(c_sl, sig_i, tanh_u)
                gate_eng.tensor_add(c_sl, c_sl, fc)
                tanh_c = work.tile([128, ssz], HDT, tag="tanh_c")
                nc.scalar.activation(tanh_c, c_sl, ACTF.Tanh)
                gate_eng.tensor_mul(h_sl, sig_o, tanh_c)

            # stores go out on the otherwise-idle gpsimd SWDGE queue: on the
            # SP HWDGE FIFO they would head-of-line block the next macro's
            # loads (stores depend on this macro's last compute).
            nc.gpsimd.dma_start(out=cT[:, m0 : m0 + msz], in_=c_out)
            nc.gpsimd.dma_start(out=hT[:, m0 : m0 + msz], in_=h_out)

    nc.compile()
    return nc


TRACE = False  # set True (e.g. from test.py) to capture an NTFF profile
LAST_RESULTS = None  # BassKernelResults of the most recent kernel() call

_PROGRAM_CACHE = {}


def _get_program(npc, in_dim, hid, cpn):
    key = (npc, in_dim, hid, cpn, MM_DT, GATES_BF16, H_F32, LOWP_FP16, HT_ON_DVE, MACRO, SUB)
    if key not in _PROGRAM_CACHE:
        _PROGRAM_CACHE[key] = build_program(npc, in_dim, hid, cpn)
    return _PROGRAM_CACHE[key]


def _pad_children(child_c, child_h, segment_ids, n):
    """Regularize to exactly max_c children per node (zero padding is exact:
    padded slots contribute sigmoid(..)*0 to fc and 0 to the child sum)."""
    seg = np.asarray(segment_ids).astype(np.int64)
    e = seg.shape[0]
    counts = np.bincount(seg, minlength=n)
    max_c = int(counts.max()) if e else 1
    if e == n * max_c and np.all(counts == max_c):
        return child_c, child_h, max_c  # already regular (and sorted)
    hid = child_h.shape[1]
    slot = np.arange(e, dtype=np.int64) - np.repeat(
        np.concatenate([[0], np.cumsum(counts)[:-1]]), counts
    )
    cc = np.zeros((n * max_c, hid), np.float32)
    ch = np.zeros((n * max_c, hid), np.float32)
    idx = seg * max_c + slot
    cc[idx] = child_c
    ch[idx] = child_h
    return cc, ch, max_c


def kernel(
    inputs,
    child_c,
    child_h,
    segment_ids,
    W_combined,
    b_combined,
    W_f,
    U_f,
    b_f,
):
    inputs = np.asarray(inputs, dtype=np.float32)
    child_c = np.asarray(child_c, dtype=np.float32)
    child_h = np.asarray(child_h, dtype=np.float32)
    n, in_dim = inputs.shape
    hid = U_f.shape[0]

    child_c, child_h, cpn = _pad_children(child_c, child_h, segment_ids, n)

    assert n % N_CORES == 0
    npc = n // N_CORES

    nc = _get_program(npc, in_dim, hid, cpn)

    import ml_dtypes

    wdt = np.float32
    if MM_DT == "bf16":
        wdt = np.float16 if LOWP_FP16 else ml_dtypes.bfloat16
    adt = wdt  # activation staging dtype (x, child_h, child_c)
    Wc = np.ascontiguousarray(np.asarray(W_combined, dtype=np.float32).astype(wdt))
    Wf = np.ascontiguousarray(np.asarray(W_f, dtype=np.float32).astype(wdt))
    Uf = np.ascontiguousarray(np.asarray(U_f, dtype=np.float32).astype(wdt))
    bc = np.ascontiguousarray(
        np.asarray(b_combined, dtype=np.float32).reshape(3, hid).T
    )
    bf = np.ascontiguousarray(np.asarray(b_f, dtype=np.float32).reshape(hid, 1))

    in_maps = []
    for c in range(N_CORES):
        n0, n1 = c * npc, (c + 1) * npc
        e0, e1 = n0 * cpn, n1 * cpn
        in_maps.append(
            {
                "xT": np.ascontiguousarray(inputs[n0:n1].T.astype(adt)),
                "chT": np.ascontiguousarray(
                    child_h[e0:e1].reshape(npc, cpn, hid).transpose(2, 1, 0).astype(adt)
                ).reshape(hid, npc * cpn),
                "ccT": np.ascontiguousarray(
                    child_c[e0:e1].reshape(npc, cpn, hid).transpose(2, 1, 0).astype(adt)
                ).reshape(hid, npc * cpn),
                "Wc": Wc,
                "Wf": Wf,
                "Uf": Uf,
                "bc": bc,
                "bf": bf,
            }
        )

    res = run_bass_kernel_spmd(
        nc, in_maps, core_ids=list(range(N_CORES)), trace=TRACE
    )
    global LAST_RESULTS
    LAST_RESULTS = res

    c_full = np.empty((n, hid), np.float32)
    h_full = np.empty((n, hid), np.float32)
    for c in range(N_CORES):
        n0, n1 = c * npc, (c + 1) * npc
        c_full[n0:n1] = res.results[c]["cT"].T.astype(np.float32)
        h_full[n0:n1] = res.results[c]["hT"].T.astype(np.float32)
    return (c_full, h_full)


if __name__ == "__main__":
    # tiny smoke test against a numpy reference
    rng = np.random.default_rng(0)
    n, in_dim, hid, cpn = 2 * N_CORES * MACRO // 25, 256, 128, 3  # small-ish
    print(f"smoke: n={n}")


# revision 30
# speedup vs baseline: 1.0185x; 1.0185x over previous
"""Batch Child-Sum TreeLSTM cell on 8 Trainium2 NeuronCores.

Strategy (data-parallel over nodes; ~231-249us/core at full size):
  - Shard the N nodes (and their contiguous child segments) evenly across the
    8 cores; replicate the small weight matrices. Irregular sorted
    segment_ids are first regularized host-side by zero-padding every node to
    max_children slots (exact: padded slots contribute 0).
  - Host stages activations feature-major ("transposed": features on SBUF
    partitions) and child-major (one contiguous slab per child slot), cast to
    fp16. Every matmul contraction runs along partitions, every moving
    operand is a contiguous slice, segment sums are plain slab adds on the
    free axis, and HBM reads halve vs fp32.
  - fp16 (not bf16): same PE/DVE throughput, 10-bit mantissa; all values here
    are O(6) so fp16 is strictly more accurate. End-to-end rel err ~2e-3.
  - PE streams each sub-tile's big_in once per z chunk (h_tilde materialized
    by two DVE fp16 2x-mode adds); forget gates are computed child-major so
    the per-edge U_f h term and the replicated W_f x term are plain matmuls.
  - ScalarE evacuates every PSUM with the fused bias+sigmoid/tanh ACTIVATE;
    VectorE does all gate arithmetic in fp16 packed mode; outputs store as
    fp16 and the host upcasts. Output stores ride the idle gpsimd SWDGE
    queue so the next macro's loads never queue behind them in the SP HWDGE
    FIFO; all three forget-gate slots accumulate into one 3-bank PSUM tile
    (512-aligned slices) evacuated by a single bias-fused sigmoid; the first
    and last macro tiles are small so compute starts and stores finish early.
    Engine busy: PE/ACT/DMA ~195-215us each, DVE ~190us.
"""

from contextlib import ExitStack

import numpy as np

import concourse.bass as bass
import concourse.bacc as bacc
import concourse.tile as tile
from concourse import mybir
from concourse.bass_utils import run_bass_kernel_spmd

F32 = mybir.dt.float32
F32R = mybir.dt.float32r
BF16 = mybir.dt.bfloat16
FP16 = mybir.dt.float16

# Matmul operand dtype. "bf16": x/child_h are cast to bf16 during the load
# DMA (SWDGE) and weights are passed as bf16 -> single-pass matmuls at full
# PE rate. "fp32r": everything stays 32-bit (2-pass matmuls, ~2x PE time)
# but ~bf16x2 precision.
MM_DT = "bf16"

# When True, the whole gate chain (activations, forget gates, cell/hidden
# assembly) and the outputs run in the 16-bit dtype: DVE tensor_tensor gets
# its 2x packed mode and the output DMA halves.
GATES_BF16 = True

# Use fp16 (not bf16) as the 16-bit dtype: same PE/DVE rates, but a 10-bit
# mantissa. All values here are O(6), nowhere near fp16 range limits, so
# fp16 is strictly more accurate than bf16 for this kernel.
LOWP_FP16 = True

# Optionally keep the h-side (sig_o, tanh_c, h) in fp32.
H_F32 = False

# Materialize h_tilde with two DVE adds instead of three extra accumulating
# matmuls per z chunk: PE streams each sub's big_in once less (15->9 MMs/sub),
# and DVE's fp16 2x mode makes the adds cheap.
HT_ON_DVE = True

N_CORES = 8

# Tiling (in nodes). MACRO: DMA / SBUF-residency granularity for x, child
# slabs and outputs. SUB: matmul moving free dim / PSUM tile width (<= 512
# fp32 per PSUM bank; keep even — walrus's fp32r dst-pattern check rejects
# odd free dims on the fp32r fallback path).
MACRO = 2500
SUB = 500


def _chunks(total, step):
    out = []
    off = 0
    while off < total:
        out.append((off, min(step, total - off)))
        off += step
    return out


def build_program(npc, in_dim, hid, cpn, engines=None):
    """Bass program for one core's shard: npc nodes, npc*cpn edges."""
    assert in_dim % 128 == 0 and hid == 128
    kx = in_dim // 128  # k-chunks of the input features
    epc = npc * cpn

    # gpsimd is poison for fine-grained work here: its EVENT_SEMAPHORE ops are
    # ucode-dispatched (~1.3us each) and dominate its engine span.
    eng = {"fjc": "vector", "fc": "vector", "gates": "vector"}
    if engines:
        eng.update(engines)

    mm_bf16 = MM_DT == "bf16"
    LP = FP16 if LOWP_FP16 else BF16  # low-precision dtype
    MMDT = LP if mm_bf16 else F32R
    WDT = LP if mm_bf16 else F32R
    GDT = LP if GATES_BF16 else F32   # gate-chain dtype
    ODT = LP if GATES_BF16 else F32   # c output dtype
    HDT = F32 if (H_F32 or not GATES_BF16) else LP  # h-side dtype

    nc = bacc.Bacc("TRN2", target_bir_lowering=False, debug=False)

    xT = nc.dram_tensor("xT", [in_dim, npc], MMDT if mm_bf16 else F32R, kind="ExternalInput").ap()
    chT = nc.dram_tensor("chT", [hid, epc], MMDT if mm_bf16 else F32R, kind="ExternalInput").ap()
    ccT = nc.dram_tensor("ccT", [hid, epc], GDT, kind="ExternalInput").ap()
    # W_combined [in+hid, 3*hid] row-major, sliced into [128,128] chunks.
    Wc = nc.dram_tensor("Wc", [in_dim + hid, 3 * hid], WDT, kind="ExternalInput").ap()
    Wf = nc.dram_tensor("Wf", [in_dim, hid], WDT, kind="ExternalInput").ap()
    Uf = nc.dram_tensor("Uf", [hid, hid], WDT, kind="ExternalInput").ap()
    bc = nc.dram_tensor("bc", [hid, 3], F32, kind="ExternalInput").ap()  # b_combined.reshape(3,128).T
    bf = nc.dram_tensor("bf", [hid, 1], F32, kind="ExternalInput").ap()

    chT3 = chT.rearrange("p (c n) -> p c n", c=cpn)
    ccT3 = ccT.rearrange("p (c n) -> p c n", c=cpn)

    cT = nc.dram_tensor("cT", [hid, npc], ODT, kind="ExternalOutput").ap()
    hT = nc.dram_tensor("hT", [hid, npc], HDT, kind="ExternalOutput").ap()

    with tile.TileContext(nc) as tc, ExitStack() as ctx:
        consts = ctx.enter_context(tc.tile_pool(name="consts", bufs=1))
        macro_pool = ctx.enter_context(tc.tile_pool(name="macro", bufs=2))
        work = ctx.enter_context(tc.tile_pool(name="work", bufs=3))
        psum = ctx.enter_context(tc.tile_pool(name="psum", bufs=2, space="PSUM"))

        # ---- weights (resident) ----
        # wc_sb[k] holds Wc rows [128k:128k+128]; k in [0, kx] are x chunks,
        # k == kx is the h_tilde chunk. Columns: 3*hid (z_i | z_o | z_u).
        wc_sb = []
        for k in range(kx + 1):
            t = consts.tile([128, 3 * hid], WDT, tag=f"wc{k}")
            nc.sync.dma_start(out=t, in_=Wc[128 * k : 128 * (k + 1), :])
            wc_sb.append(t)
        wf_sb = []
        for k in range(kx):
            t = consts.tile([128, hid], WDT, tag=f"wf{k}")
            nc.sync.dma_start(out=t, in_=Wf[128 * k : 128 * (k + 1), :])
            wf_sb.append(t)
        uf_sb = consts.tile([128, hid], WDT, tag="uf")
        nc.sync.dma_start(out=uf_sb, in_=Uf)
        bc_sb = consts.tile([128, 3], F32, tag="bc")
        nc.sync.dma_start(out=bc_sb, in_=bc)
        bf_sb = consts.tile([128, 1], F32, tag="bf")
        nc.sync.dma_start(out=bf_sb, in_=bf)

        ACTF = mybir.ActivationFunctionType
        # Shared 3-bank PSUM tile for all forget-gate slots (single sigmoid
        # evacuation). Allocated once (bufs=1 semantics) and memset so the
        # pad columns between the slots are never read uninitialized.
        fp_all = None
        if cpn >= 2 and SUB <= 512 and cpn <= 3:
            fp_all = psum.tile([128, 512 * cpn], F32, tag="fp_all", bufs=1)
            nc.vector.memset(fp_all, 0.0)
        fjc_eng = getattr(nc, eng["fjc"])
        fc_eng = getattr(nc, eng["fc"])
        gate_eng = getattr(nc, eng["gates"])

        macro_plan = []
        if npc > 2 * MACRO:
            # small first macro (compute starts early) and small last macro
            # (stores start early, short tail)
            macro_plan.append((0, SUB))
            macro_plan += [(SUB + o, s) for o, s in _chunks(npc - 2 * SUB, MACRO)]
            macro_plan.append((npc - SUB, SUB))
        else:
            macro_plan = _chunks(npc, MACRO)
        for m0, msz in macro_plan:
            x_sb = []
            for k in range(kx):
                t = macro_pool.tile([128, msz], MMDT, tag=f"x{k}", bufs=3)
                nc.sync.dma_start(out=t, in_=xT[128 * k : 128 * (k + 1), m0 : m0 + msz])
                x_sb.append(t)
            c_out = macro_pool.tile([128, msz], ODT, tag="c_out")
            h_out = macro_pool.tile([128, msz], HDT, tag="h_out")

            # child tensors: one contiguous DMA per child slab per macro
            ch_mac = macro_pool.tile([128, cpn * msz], MMDT, tag="ch", bufs=3)
            cc_mac = macro_pool.tile([128, cpn * msz], GDT, tag="cc", bufs=3)
            for c in range(cpn):
                nc.sync.dma_start(
                    out=ch_mac[:, c * msz : (c + 1) * msz],
                    in_=chT3[:, c, m0 : m0 + msz],
                )
                nc.sync.dma_start(
                    out=cc_mac[:, c * msz : (c + 1) * msz],
                    in_=ccT3[:, c, m0 : m0 + msz],
                )
            ch_mac3 = ch_mac.rearrange("p (c n) -> p c n", c=cpn)
            cc_mac3 = cc_mac.rearrange("p (c n) -> p c n", c=cpn)

            for s0, ssz in _chunks(msz, SUB):
                n0 = m0 + s0  # absolute node offset
                esz = ssz * cpn

                ch3 = ch_mac3[:, :, s0 : s0 + ssz]
                cc3 = cc_mac3[:, :, s0 : s0 + ssz]

                if HT_ON_DVE and cpn > 1:
                    ht = work.tile([128, ssz], MMDT, tag="ht")
                    nc.vector.tensor_add(ht, ch3[:, 0, :], ch3[:, 1, :])
                    for ci in range(2, cpn):
                        nc.vector.tensor_add(ht, ht, ch3[:, ci, :])
                elif HT_ON_DVE:
                    ht = ch3[:, 0, :]

                # ---- z = big_in @ W_combined + b_combined (transposed) ----
                zp = []
                for j in range(3):  # z_i, z_o, z_u output chunks
                    p = psum.tile([128, ssz], F32, tag=f"z{j}", bufs=(1 if j == 2 else 2))
                    for k in range(kx):
                        nc.tensor.matmul(
                            p,
                            lhsT=wc_sb[k][:, 128 * j : 128 * (j + 1)],
                            rhs=x_sb[k][:, s0 : s0 + ssz],
                            start=(k == 0),
                            stop=False,
                        )
                    if HT_ON_DVE:
                        nc.tensor.matmul(
                            p,
                            lhsT=wc_sb[kx][:, 128 * j : 128 * (j + 1)],
                            rhs=ht,
                            start=False,
                            stop=True,
                        )
                    else:
                        for c in range(cpn):  # += W_h^T @ h_tilde^T
                            nc.tensor.matmul(
                                p,
                                lhsT=wc_sb[kx][:, 128 * j : 128 * (j + 1)],
                                rhs=ch3[:, c, :],
                                start=False,
                                stop=(c == cpn - 1),
                            )
                    zp.append(p)

                sig_i = work.tile([128, ssz], GDT, tag="sig_i")
                nc.scalar.activation(sig_i, zp[0], ACTF.Sigmoid, bias=bc_sb[:, 0:1])
                sig_o = work.tile([128, ssz], HDT, tag="sig_o")
                nc.scalar.activation(sig_o, zp[1], ACTF.Sigmoid, bias=bc_sb[:, 1:2])
                tanh_u = work.tile([128, ssz], GDT, tag="tanh_u")
                nc.scalar.activation(tanh_u, zp[2], ACTF.Tanh, bias=bc_sb[:, 2:3])

                # ---- f_jk = sigmoid(W_f x_j + U_f h_k + b_f), child-major ----
                # One PSUM tile per child slot c: f_c = U_f^T ch[:, c::cpn]
                # + W_f^T x (x term identical for all slots, plain slices —
                # no broadcast APs, which the fp32r ISA checker rejects).
                merged = fp_all is not None and ssz <= 512
                fw = 512 * (cpn - 1) + ssz
                if merged:
                    f_all = work.tile([128, fw], GDT, tag="f_all")
                f_slices = []
                for c in range(cpn):
                    if merged:
                        fp = fp_all[:, 512 * c : 512 * c + ssz]
                    else:
                        fp = psum.tile([128, ssz], F32, tag="fp", bufs=1)
                    nc.tensor.matmul(
                        fp, lhsT=uf_sb, rhs=ch3[:, c, :], start=True, stop=False
                    )
                    for k in range(kx):
                        nc.tensor.matmul(
                            fp,
                            lhsT=wf_sb[k],
                            rhs=x_sb[k][:, s0 : s0 + ssz],
                            start=False,
                            stop=(k == kx - 1),
                        )
                    if merged:
                        f_slices.append(f_all[:, 512 * c : 512 * c + ssz])
                        if c == cpn - 1:
                            # single sigmoid evacuates every slot (same b_f
                            # bias); pad columns hold garbage nobody reads
                            nc.scalar.activation(
                                f_all, fp_all[:, 0:fw], ACTF.Sigmoid,
                                bias=bf_sb[:, 0:1],
                            )
                        continue
                    f_c = work.tile([128, ssz], GDT, tag=f"f{c}")
                    nc.scalar.activation(f_c, fp, ACTF.Sigmoid, bias=bf_sb[:, 0:1])
                    f_slices.append(f_c)
                fjc_c = []
                for c in range(cpn):
                    fjc = work.tile([128, ssz], GDT, tag=f"fjc{c}")
                    fjc_eng.tensor_mul(fjc, f_slices[c], cc3[:, c, :])
                    fjc_c.append(fjc)

                # ---- fc = segment_sum(f * child_c) ----
                if cpn == 1:
                    fc = fjc_c[0]
                else:
                    fc = work.tile([128, ssz], GDT, tag="fc")
                    fc_eng.tensor_add(fc, fjc_c[0], fjc_c[1])
                    for ci in range(2, cpn):
                        fc_eng.tensor_add(fc, fc, fjc_c[ci])

                # ---- c, h ----
                c_sl = c_out[:, s0 : s0 + ssz]
                h_sl = h_out[:, s0 : s0 + ssz]
                gate_eng.tensor_mul(c_sl, sig_i, tanh_u)
                gate_eng.tensor_add(c_sl, c_sl, fc)
                tanh_c = work.tile([128, ssz], HDT, tag="tanh_c")
                nc.scalar.activation(tanh_c, c_sl, ACTF.Tanh)
                gate_eng.tensor_mul(h_sl, sig_o, tanh_c)

            # stores go out on the otherwise-idle gpsimd SWDGE queue: on the
            # SP HWDGE FIFO they would head-of-line block the next macro's
            # loads (stores depend on this macro's last compute).
            nc.gpsimd.dma_start(out=cT[:, m0 : m0 + msz], in_=c_out)
            nc.gpsimd.dma_start(out=hT[:, m0 : m0 + msz], in_=h_out)

    nc.compile()
    return nc


TRACE = False  # set True (e.g. from test.py) to capture an NTFF profile
LAST_RESULTS = None  # BassKernelResults of the most recent kernel() call

_PROGRAM_CACHE = {}


def _get_program(npc, in_dim, hid, cpn):
    key = (npc, in_dim, hid, cpn, MM_DT, GATES_BF16, H_F32, LOWP_FP16, HT_ON_DVE, MACRO, SUB)
    if key not in _PROGRAM_CACHE:
        _PROGRAM_CACHE[key] = build_program(npc, in_dim, hid, cpn)
    return _PROGRAM_CACHE[key]


def _pad_children(child_c, child_h, segment_ids, n):
    """Regularize to exactly max_c children per node (zero padding is exact:
    padded slots contribute sigmoid(..)*0 to fc and 0 to the child sum)."""
    seg = np.asarray(segment_ids).astype(np.int64)
    e = seg.shape[0]
    counts = np.bincount(seg, minlength=n)
    max_c = int(counts.max()) if e else 1
    if e == n * max_c and np.all(counts == max_c):
        return child_c, child_h, max_c  # already regular (and sorted)
    hid = child_h.shape[1]
    slot = np.arange(e, dtype=np.int64) - np.repeat(
        np.concatenate([[0], np.cumsum(counts)[:-1]]), counts
    )
    cc = np.zeros((n * max_c, hid), np.float32)
    ch = np.zeros((n * max_c, hid), np.float32)
    idx = seg * max_c + slot
    cc[idx] = child_c
    ch[idx] = child_h
    return cc, ch, max_c


def kernel(
    inputs,
    child_c,
    child_h,
    segment_ids,
    W_combined,
    b_combined,
    W_f,
    U_f,
    b_f,
):
    inputs = np.asarray(inputs, dtype=np.float32)
    child_c = np.asarray(child_c, dtype=np.float32)
    child_h = np.asarray(child_h, dtype=np.float32)
    n, in_dim = inputs.shape
    hid = U_f.shape[0]

    child_c, child_h, cpn = _pad_children(child_c, child_h, segment_ids, n)

    assert n % N_CORES == 0
    npc = n // N_CORES

    nc = _get_program(npc, in_dim, hid, cpn)

    import ml_dtypes

    wdt = np.float32
    if MM_DT == "bf16":
        wdt = np.float16 if LOWP_FP16 else ml_dtypes.bfloat16
    adt = wdt  # activation staging dtype (x, child_h, child_c)
    Wc = np.ascontiguousarray(np.asarray(W_combined, dtype=np.float32).astype(wdt))
    Wf = np.ascontiguousarray(np.asarray(W_f, dtype=np.float32).astype(wdt))
    Uf = np.ascontiguousarray(np.asarray(U_f, dtype=np.float32).astype(wdt))
    bc = np.ascontiguousarray(
        np.asarray(b_combined, dtype=np.float32).reshape(3, hid).T
    )
    bf = np.ascontiguousarray(np.asarray(b_f, dtype=np.float32).reshape(hid, 1))

    in_maps = []
    for c in range(N_CORES):
        n0, n1 = c * npc, (c + 1) * npc
        e0, e1 = n0 * cpn, n1 * cpn
        in_maps.append(
            {
                "xT": np.ascontiguousarray(inputs[n0:n1].T.astype(adt)),
                "chT": np.ascontiguousarray(
                    child_h[e0:e1].reshape(npc, cpn, hid).transpose(2, 1, 0).astype(adt)
                ).reshape(hid, npc * cpn),
                "ccT": np.ascontiguousarray(
                    child_c[e0:e1].reshape(npc, cpn, hid).transpose(2, 1, 0).astype(adt)
                ).reshape(hid, npc * cpn),
                "Wc": Wc,
                "Wf": Wf,
                "Uf": Uf,
                "bc": bc,
                "bf": bf,
            }
        )

    res = run_bass_kernel_spmd(
        nc, in_maps, core_ids=list(range(N_CORES)), trace=TRACE
    )
    global LAST_RESULTS
    LAST_RESULTS = res

    c_full = np.empty((n, hid), np.float32)
    h_full = np.empty((n, hid), np.float32)
    for c in range(N_CORES):
        n0, n1 = c * npc, (c + 1) * npc
        c_full[n0:n1] = res.results[c]["cT"].T.astype(np.float32)
        h_full[n0:n1] = res.results[c]["hT"].T.astype(np.float32)
    return (c_full, h_full)


if __name__ == "__main__":
    # tiny smoke test against a numpy reference
    rng = np.random.default_rng(0)
    n, in_dim, hid, cpn = 2 * N_CORES * MACRO // 25, 256, 128, 3  # small-ish
    print(f"smoke: n={n}")
